# revision 1
# baseline (speedup 1.0000x reference)
"""Trainium2 Bass kernel for DeepConvWeightNet (3-layer CNN body + 3 CA-heads
+ box-blur fusion), data-parallel over 8 NeuronCores (1 image per core).

Self-contained: imports only numpy + the concourse stack at /opt/trn_rl_repo.
"""

import json
import os
import sys

import numpy as np

for _p in ("/opt/trn_rl_repo", "/root/.axon_site/_ro/trn_rl_repo"):
    if os.path.isdir(_p) and _p not in sys.path:
        sys.path.insert(0, _p)
        break

import concourse.bass as bass
from bass_rust import AP as _RawAP
import concourse.tile as tile
from concourse import mybir
from concourse.bass_utils import run_bass_kernel_spmd

# ---- NEFF cache: walrus BIR->NEFF compile is ~3 min and deterministic per
# BIR; cache the result on disk so repeat processes (and the 8 worker procs)
# compile once. Patch the name bass2jax's neuronx_cc_hook resolves.
_NEFF_CACHE_DIR = os.environ.get("KERNEL_NEFF_CACHE",
                                 "/tmp/kernel_neff_cache")


def _install_neff_cache():
    import hashlib
    import shutil
    import tempfile
    import time as _time

    from concourse import bass2jax as _b2j
    if getattr(_b2j, "_neff_cache_installed", False):
        return
    # libneuronxla's HLO-keyed NEFF cache serves stale executables for
    # bass_exec modules (the hash collides across different embedded BIRs),
    # silently skipping our compile hook. Point it at a throwaway dir.
    os.environ["NEURON_COMPILE_CACHE_URL"] = tempfile.mkdtemp(
        prefix="neuron-cc-nocache-")
    _orig_compile = _b2j.compile_bir_kernel

    def _cached_compile(bir_json, tmpdir, neff_name="file.neff"):
        os.makedirs(_NEFF_CACHE_DIR, exist_ok=True)
        key = hashlib.sha256(bir_json).hexdigest()[:32]
        cpath = os.path.join(_NEFF_CACHE_DIR, key + ".neff")
        lpath = os.path.join(_NEFF_CACHE_DIR, key + ".lock")
        if not os.path.exists(cpath):
            # one process compiles; others (workers) wait on the lock
            try:
                fd = os.open(lpath, os.O_CREAT | os.O_EXCL | os.O_WRONLY)
            except FileExistsError:
                fd = None
            if fd is not None:
                try:
                    out = _orig_compile(bir_json, tmpdir, neff_name)
                    shutil.copy(out, cpath + ".part")
                    os.replace(cpath + ".part", cpath)
                finally:
                    os.close(fd)
                    os.unlink(lpath)
            else:
                for _ in range(7200):
                    if os.path.exists(cpath):
                        break
                    _time.sleep(0.5)
                else:
                    raise TimeoutError("NEFF cache wait timed out")
        dst = os.path.join(tmpdir, neff_name)
        shutil.copy(cpath, dst)
        return dst

    _b2j.compile_bir_kernel = _cached_compile
    _b2j._neff_cache_installed = True

F32 = mybir.dt.float32
F32R = mybir.dt.float32r
BF16 = mybir.dt.bfloat16
F16 = mybir.dt.float16

H = W = 512
NPIX = H * W
PADW = 536     # fuse-land padded width  (12 halo each side)
PADH = 544     # fuse-land padded height (12 top, 20 bottom)
PW1 = 514      # conv-land padded width/height (1 halo each side)
BLURS = [(5, 2), (15, 7), (25, 12)]   # (k, radius)
FT = 104       # fuse tile rows
NFT = 5        # number of fuse tiles (4*104 + 96)

# conv datapath dtype: "f32r" (fp32 storage, relaxed matmul) or "bf16"
DT_CONV = os.environ.get("KERNEL_DT", "f32r")
# A/B debug switches for the compact-I/O paths. IEEE f16 SBUF->DRAM output
# tiles corrupt in-context on this silicon (even-column garbage in the last
# partition chunk); bf16 output is the native, safe 2-byte path.
_X_F16 = os.environ.get("KERNEL_X16", "1") == "1"
_OUT_DT = os.environ.get("KERNEL_OUTDT", "f32")   # bf16 | f16 | f32


def _taps():
    return [(di, dj) for di in (-1, 0, 1) for dj in (-1, 0, 1)]


# ---------------------------------------------------------------- BIR postpass
def _split_waits_json(bir: bytes, limit: int = 1) -> bytes:
    """walrus (this snapshot) rejects >1 sync-wait per instruction; spill
    extra waits onto same-engine Drain carriers inserted just before."""
    d = json.loads(bir)
    n = [0]
    for f in d.get("functions", []):
        for bb in f.get("blocks", []):
            out = []
            for inst in bb.get("instructions", []):
                si = inst.get("sync_info") or {}
                ow = si.get("on_wait") or []
                if len(ow) > limit:
                    chunks = [ow[i:i + limit] for i in range(0, len(ow), limit)]
                    si["on_wait"] = chunks[-1]
                    for ch in chunks[:-1]:
                        n[0] += 1
                        out.append({
                            "debug": inst.get("debug", 0),
                            "engine": inst["engine"],
                            "ins": [], "outs": [],
                            "is_reset_sema": False,
                            "name": f"I-ws-{n[0]}",
                            "opcode": "Drain",
                            "sync_info": {"on_update": [], "on_wait": ch},
                        })
                out.append(inst)
            bb["instructions"] = out
    return json.dumps(d).encode()


# ------------------------------------------------------------------- builders

_DMA_RR = [0]

def _dma(nc, out, in_):
    """Round-robin big DMAs across engines so transfers spread over queues."""
    engs = (nc.sync, nc.gpsimd, nc.scalar, nc.sync, nc.gpsimd,
            nc.scalar, nc.sync, nc.gpsimd)
    e = engs[_DMA_RR[0] % len(engs)]
    _DMA_RR[0] += 1
    return e.dma_start(out=out, in_=in_)


def _conv_layer(nc, tc, ctx, *, src, dst, weights, bias_t, cin_parts, cout,
                th, n_dj, alpha, dt_conv, alpha_f=0.0, gap_t=None, src_off=(1, 1),
                src_stride=PW1, dst_kind="pad"):
    """One 3x3 conv layer, K-packed matmuls.

    src: DRAM tensor [Cg*32?, rows, src_stride] padded input (1-halo origin at
         src_off). weights: list over (half, dj) of SBUF lhsT tiles, given as
         list of lists: weights[half][dj]; each half reads 32*len-partition
         groups built from channel slice `half`.
    dst: DRAM tensor; "pad" -> [cout, PW1, PW1] interior write at (1,1);
         "flat" -> [cout, H, W].
    """
    n_half = len(weights)
    mmdt = F32R if dt_conv == "f32r" else BF16
    xdt = mmdt
    odt = F32 if dst_kind == "flat" else mmdt
    G = 4                      # psum rows per bank-group
    xp = ctx.enter_context(tc.tile_pool(name=f"x_{dst.name}", bufs=2))
    op = ctx.enter_context(tc.tile_pool(name=f"o_{dst.name}", bufs=2))
    pp = ctx.enter_context(tc.tile_pool(name=f"p_{dst.name}", bufs=2, space="PSUM"))
    r0, c0 = src_off
    for t0 in range(0, H, th):
        xs = []
        for half in range(n_half):
            xt = xp.tile([96, th * src_stride], xdt, tag=f"x{half}")
            plane = src.shape[1] * src_stride
            _dma(nc, out=xt[0:96, 0:th * src_stride],
                 in_=_RawAP(tensor=src[:].tensor,
                            offset=32 * half * plane + ((r0 - 1) + t0) * src_stride,
                            ap=[[src_stride, 3], [plane, 32],
                                [src_stride, th], [1, src_stride]]))
            xs.append(xt)
        ot = op.tile([cout, th * 512], odt)
        for rg in range(th // G):
            pt = pp.tile([cout, G * 512], F32)
            for half in range(n_half):
                for dj in range(n_dj):
                    for r4 in range(G):
                        r = rg * G + r4
                        nc.tensor.matmul(
                            pt[:, r4 * 512:(r4 + 1) * 512],
                            weights[half][dj][:],
                            xs[half][0:96, r * src_stride + (c0 - 1) + dj:
                                     r * src_stride + (c0 - 1) + dj + 512],
                            start=(half == 0 and dj == 0),
                            stop=(half == n_half - 1 and dj == n_dj - 1),
                        )
            osl = ot[:, rg * G * 512:(rg + 1) * G * 512]
            if alpha is not None:
                nc.scalar.activation(osl[:, 0:1536], pt[:, 0:1536],
                                     mybir.ActivationFunctionType.Prelu,
                                     bias=bias_t[0:cout, 0:1], alpha=alpha[0:cout, 0:1])
                ptmp = op.tile([cout, 512], F32, tag="ptmp", name="ptmp")
                nc.vector.tensor_scalar(out=ptmp[:], in0=pt[:, 1536:2048],
                                        scalar1=bias_t[0:cout, 0:1], scalar2=None,
                                        op0=mybir.AluOpType.add)
                nc.vector.scalar_tensor_tensor(
                    out=osl[:, 1536:2048], in0=ptmp[:], scalar=alpha_f, in1=ptmp[:],
                    op0=mybir.AluOpType.mult, op1=mybir.AluOpType.max)
            else:
                e = len(gap_t[1])
                gap_t[1].append(e)
                nc.vector.tensor_scalar(
                    out=osl, in0=pt[:, :], scalar1=bias_t[0:cout, 0:1],
                    scalar2=0.0, op0=mybir.AluOpType.add,
                    op1=mybir.AluOpType.add,
                    accum_out=gap_t[0][0:cout, e:e + 1])
        if dst_kind == "pad":
            _dma(nc, out=dst[0:cout, 1 + t0:1 + t0 + th, 1:513],
                 in_=ot.rearrange("c (h w) -> c h w", w=512))
        else:
            _dma(nc, out=dst[0:cout, t0:t0 + th, 0:512],
                 in_=ot.rearrange("c (h w) -> c h w", w=512))


def _conv1(nc, tc, ctx, *, xpad, dst, w1_t, bias_t, alpha, th, alpha_f=0.0):
    xp = ctx.enter_context(tc.tile_pool(name="x_c1", bufs=2))
    op = ctx.enter_context(tc.tile_pool(name="o_c1", bufs=2))
    pp = ctx.enter_context(tc.tile_pool(name="p_c1", bufs=2, space="PSUM"))
    G = 4
    for t0 in range(0, H, th):
        xt = xp.tile([27, th * PADW], F32R, tag="x")
        for di in range(3):
            _dma(nc, out=xt[9 * di:9 * di + 9, 0:th * PADW],
                 in_=_RawAP(tensor=xpad[:].bitcast(F32R).tensor,
                            offset=(11 + t0 + di) * PADW,
                            ap=[[1, 3], [PADH * PADW, 3], [1, th * PADW]]))
        ot = op.tile([32, th * 512], F32R)
        for rg in range(th // G):
            pt = pp.tile([32, G * 512], F32)
            for r4 in range(G):
                r = rg * G + r4
                nc.tensor.matmul(
                    pt[:, r4 * 512:(r4 + 1) * 512], w1_t[:],
                    xt[0:27, r * PADW + 11:r * PADW + 11 + 512],
                    start=True, stop=True)
            osl = ot[:, rg * G * 512:(rg + 1) * G * 512]
            nc.scalar.activation(osl[:, 0:1536], pt[:, 0:1536],
                                 mybir.ActivationFunctionType.Prelu,
                                 bias=bias_t[0:32, 0:1], alpha=alpha[0:32, 0:1])
            ptmp = op.tile([32, 512], F32, tag="ptmp", name="ptmp")
            nc.vector.tensor_scalar(out=ptmp[:], in0=pt[:, 1536:2048],
                                    scalar1=bias_t[0:32, 0:1], scalar2=None,
                                    op0=mybir.AluOpType.add)
            nc.vector.scalar_tensor_tensor(
                out=osl[:, 1536:2048], in0=ptmp[:], scalar=alpha_f, in1=ptmp[:],
                op0=mybir.AluOpType.mult, op1=mybir.AluOpType.max)
        _dma(nc, out=dst[0:32, 1 + t0:1 + t0 + th, 1:513],
             in_=ot.rearrange("c (h w) -> c h w", w=512))


def build_nc(a_vals, dt_conv):
    nc = bass.Bass()
    dtw = F32R if dt_conv == "f32r" else BF16
    wdt_decl = "float32" if dt_conv == "f32r" else "bfloat16"

    # ---- external params (x/out ship as f16 — tunnel bandwidth is the
    # system bottleneck; converted to/from f32 on device)
    x_in = nc.declare_dram_parameter("x", [3, H, W],
                                     F16 if _X_F16 else F32, isOutput=False)
    w1 = nc.declare_dram_parameter("w1", [27, 32], dtw, isOutput=False)
    w2 = nc.declare_dram_parameter("w2", [3, 96, 64], dtw, isOutput=False)
    w3 = nc.declare_dram_parameter("w3", [2, 3, 96, 32], dtw, isOutput=False)
    wh = nc.declare_dram_parameter("wh", [3, 96, 12], dtw, isOutput=False)
    bb1 = nc.declare_dram_parameter("bb1", [32, 1], F32, isOutput=False)
    bb2 = nc.declare_dram_parameter("bb2", [64, 1], F32, isOutput=False)
    bb3 = nc.declare_dram_parameter("bb3", [32, 1], F32, isOutput=False)
    hbb = nc.declare_dram_parameter("hbb", [12, 1], F32, isOutput=False)
    c1t = nc.declare_dram_parameter("c1t", [3, 4, 4], F32, isOutput=False)
    c2t = nc.declare_dram_parameter("c2t", [3, 4, 4], F32, isOutput=False)
    bnd = nc.declare_dram_parameter("bnd", [3, 128, FT], dtw, isOutput=False)
    # 16-bit output mode ships packed pairs in f32 words: 16-bit SBUF->DRAM
    # DMAs corrupt their tail chunk on this silicon, f32 DMAs don't.
    if _OUT_DT == "f32":
        out = nc.declare_dram_parameter("out", [3, H, W], F32, isOutput=True)
    else:
        out = nc.declare_dram_parameter("out", [3, H, W // 2], F32,
                                        isOutput=True)
    dbg = os.environ.get("KERNEL_DEBUG") == "1"
    if dbg:
        d_b1 = nc.declare_dram_parameter("d_b1", [32, PW1, PW1], F32, isOutput=True)
        d_b3 = nc.declare_dram_parameter("d_b3", [32, PW1, PW1], F32, isOutput=True)
        d_hpre = nc.declare_dram_parameter("d_hpre", [12, H, W], F32, isOutput=True)
        d_sball = nc.declare_dram_parameter("d_sball", [128, 12], F32, isOutput=True)
        d_fu1 = nc.declare_dram_parameter("d_fu1", [3, PADH, PADW], F32, isOutput=True)

    # ---- DRAM scratch
    bdt = F32R if dt_conv == "f32r" else BF16
    xpad = nc.dram_tensor("xpad", [3, PADH, PADW], F32)
    b1 = nc.dram_tensor("b1", [32, PW1, PW1], bdt)
    b2 = nc.dram_tensor("b2", [64, PW1, PW1], bdt)
    b3 = nc.dram_tensor("b3", [32, PW1, PW1], bdt)
    hpre = nc.dram_tensor("hpre", [12, H, W], F32)
    fu1 = nc.dram_tensor("fu1", [3, PADH, PADW], F32)
    fu2 = nc.dram_tensor("fu2", [3, PADH, PADW], F32)

    with tile.TileContext(nc) as tc:
        from contextlib import ExitStack
        with ExitStack() as top:
            cpool = top.enter_context(tc.tile_pool(name="consts", bufs=1))

            # ---- load consts
            w1_t = cpool.tile([27, 32], dtw)
            nc.sync.dma_start(out=w1_t[:], in_=w1[:])
            w2_t = [cpool.tile([96, 64], dtw, tag=f"w2{j}", name=f"w2_{j}") for j in range(3)]
            for j in range(3):
                nc.sync.dma_start(out=w2_t[j][:], in_=w2[j])
            w3_t = [[cpool.tile([96, 32], dtw, tag=f"w3{h}{j}", name=f"w3_{h}_{j}") for j in range(3)]
                    for h in range(2)]
            for h in range(2):
                for j in range(3):
                    nc.sync.dma_start(out=w3_t[h][j][:], in_=w3[h, j])
            wh_t = [cpool.tile([96, 12], dtw, tag=f"wh{j}", name=f"wh_{j}") for j in range(3)]
            for j in range(3):
                nc.sync.dma_start(out=wh_t[j][:], in_=wh[j])
            bb1_t = cpool.tile([32, 1], F32); nc.sync.dma_start(out=bb1_t[:], in_=bb1[:])
            bb2_t = cpool.tile([64, 1], F32); nc.sync.dma_start(out=bb2_t[:], in_=bb2[:])
            bb3_t = cpool.tile([32, 1], F32); nc.sync.dma_start(out=bb3_t[:], in_=bb3[:])
            hbb_t = cpool.tile([12, 1], F32); nc.sync.dma_start(out=hbb_t[:], in_=hbb[:])
            c1_t = [cpool.tile([4, 4], F32, tag=f"c1{h}", name=f"c1_{h}") for h in range(3)]
            c2_t = [cpool.tile([4, 4], F32, tag=f"c2{h}", name=f"c2_{h}") for h in range(3)]
            for h in range(3):
                nc.sync.dma_start(out=c1_t[h][:], in_=c1t[h])
                nc.sync.dma_start(out=c2_t[h][:], in_=c2t[h])
            bnd_t = [cpool.tile([128, FT], dtw, tag=f"bnd{k}", name=f"bnd_{k}") for k in range(3)]
            for k in range(3):
                nc.sync.dma_start(out=bnd_t[k][:], in_=bnd[k])
            al_t = []
            for i in range(3):
                at = cpool.tile([128, 1], F32, tag=f"al{i}", name=f"al_{i}")
                nc.gpsimd.memset(at[:], float(a_vals[i]))
                al_t.append(at)
            ones_t = cpool.tile([1, 128], F32)
            nc.gpsimd.memset(ones_t[:], 1.0)
            sball = cpool.tile([128, 12], F32)
            gap_t = cpool.tile([12, 128], F32)

            # ---- zero borders + x interior
            with tc.tile_pool(name="zinit", bufs=1) as zp:
                z = zp.tile([128, 6144], F32)
                nc.gpsimd.memset(z[:], 0.0)
                for buf in (xpad, fu1, fu2):
                    for r in range(12):
                        nc.sync.dma_start(out=buf[0:3, r, 0:PADW], in_=z[0:3, 0:PADW])
                    for r in range(524, PADH):
                        nc.sync.dma_start(out=buf[0:3, r, 0:PADW], in_=z[0:3, 0:PADW])
                    nc.sync.dma_start(
                        out=buf[0:3, 12:524, 0:12],
                        in_=z[0:3, 0:512 * 12].rearrange("c (a b) -> c a b", b=12))
                    nc.sync.dma_start(
                        out=buf[0:3, 12:524, 524:PADW],
                        in_=z[0:3, 0:512 * 12].rearrange("c (a b) -> c a b", b=12))
                zb = zp.tile([128, 6144], F32 if dt_conv == "f32r" else BF16)
                nc.gpsimd.memset(zb[:], 0.0)
                zc = (lambda ap: ap.bitcast(F32R)) if dt_conv == "f32r" else (lambda ap: ap)
                for buf, cc in ((b1, 32), (b2, 64), (b3, 32)):
                    nc.sync.dma_start(out=buf[0:cc, 0, 0:PW1], in_=zc(zb[0:cc, 0:PW1]))
                    nc.sync.dma_start(out=buf[0:cc, 513, 0:PW1], in_=zc(zb[0:cc, 0:PW1]))
                    nc.sync.dma_start(out=buf[0:cc, 1:513, 0:1],
                                      in_=zc(zb[0:cc, 0:512].rearrange("c (a b) -> c a b", b=1)))
                    nc.sync.dma_start(out=buf[0:cc, 1:513, 513:PW1],
                                      in_=zc(zb[0:cc, 0:512].rearrange("c (a b) -> c a b", b=1)))
                if _X_F16:
                    for b in range(12):
                        ch, r0 = b // 4, (b % 4) * 128
                        t16 = zp.tile([128, 512], F16, tag="xc16")
                        nc.sync.dma_start(out=t16[:],
                                          in_=x_in[ch, r0:r0 + 128, 0:512])
                        t32 = zp.tile([128, 512], F32, tag="xc32")
                        nc.gpsimd.tensor_copy(t32[:], t16[:])
                        nc.sync.dma_start(
                            out=xpad[ch, 12 + r0:12 + r0 + 128, 12:524],
                            in_=t32[:])
                else:
                    nc.sync.dma_start(out=xpad[0:3, 12:524, 12:524],
                                      in_=x_in[:])

            # ---- conv stack
            with ExitStack() as ph:
                _conv1(nc, tc, ctx=ph, xpad=xpad, dst=b1, w1_t=w1_t,
                       bias_t=bb1_t, alpha=al_t[0], th=16,
                       alpha_f=float(a_vals[0]))
            with ExitStack() as ph:
                _conv_layer(nc, tc, ph, src=b1, dst=b2, weights=[w2_t],
                            bias_t=bb2_t, cin_parts=96, cout=64, th=16, n_dj=3,
                            alpha=al_t[1], alpha_f=float(a_vals[1]), dt_conv=dt_conv)
            with ExitStack() as ph:
                _conv_layer(nc, tc, ph, src=b2, dst=b3, weights=w3_t,
                            bias_t=bb3_t, cin_parts=96, cout=32, th=8, n_dj=3,
                            alpha=al_t[2], alpha_f=float(a_vals[2]), dt_conv=dt_conv)
            gap_cols = []
            with ExitStack() as ph:
                _conv_layer(nc, tc, ph, src=b3, dst=hpre, weights=[wh_t],
                            bias_t=hbb_t, cin_parts=96, cout=12, th=16, n_dj=3,
                            alpha=None, dt_conv=dt_conv,
                            gap_t=(gap_t, gap_cols), dst_kind="flat")

            # ---- CA layer -> sball [128, 12]
            with ExitStack() as ph:
                cap = ph.enter_context(tc.tile_pool(name="ca", bufs=1))
                cps = ph.enter_context(tc.tile_pool(name="caps", bufs=2, space="PSUM"))
                gsum = cap.tile([12, 1], F32)
                nc.vector.tensor_reduce(
                    out=gsum[:], in_=gap_t[0:12, 0:len(gap_cols)],
                    axis=mybir.AxisListType.X, op=mybir.AluOpType.add)
                for h in range(3):
                    gh = cap.tile([4, 1], F32, tag="gh")
                    nc.sync.dma_start(out=gh[:], in_=gsum[4 * h:4 * h + 4, 0:1])
                    p1 = cps.tile([4, 512], F32, tag="cp")
                    nc.tensor.matmul(p1[0:4, 0:1], c1_t[h][:], gh[:],
                                     start=True, stop=True)
                    r1 = cap.tile([4, 1], F32, tag="r1")
                    nc.scalar.activation(r1[:], p1[0:4, 0:1],
                                         mybir.ActivationFunctionType.Relu)
                    p2 = cps.tile([4, 512], F32, tag="cp")
                    nc.tensor.matmul(p2[0:4, 0:1], c2_t[h][:], r1[:],
                                     start=True, stop=True)
                    sh = cap.tile([4, 1], F32, tag="sh")
                    nc.scalar.activation(sh[:], p2[0:4, 0:1],
                                         mybir.ActivationFunctionType.Sigmoid)
                    sT = cap.tile([1, 4], F32, tag="sT")
                    nc.sync.dma_start(out=sT[:], in_=sh[:])
                    for c in range(4):
                        pb = cps.tile([128, 512], F32, tag="cb")
                        nc.tensor.matmul(pb[0:128, 0:1], ones_t[:],
                                         sT[0:1, c:c + 1], start=True, stop=True)
                        nc.vector.tensor_copy(sball[:, 4 * h + c:4 * h + c + 1],
                                              pb[0:128, 0:1])

            if dbg:
                with tc.tile_pool(name="dbgp", bufs=1) as dp:
                    dt_ = dp.tile([128, 12], F32)
                    nc.vector.tensor_copy(dt_[:], sball[:])
                    nc.sync.dma_start(out=d_sball[:], in_=dt_[:])
                nc.sync.dma_start(out=d_b1[:], in_=b1[:].bitcast(F32))
                nc.sync.dma_start(out=d_b3[:], in_=b3[:].bitcast(F32))
                nc.sync.dma_start(out=d_hpre[:], in_=hpre[:])

            # ---- fuse stages
            stages = [(xpad, fu1), (fu1, fu2), (fu2, None)]
            _post_fuse_dbg = dbg
            for s, (cur, dstb) in enumerate(stages):
                with ExitStack() as ph:
                    fp = ph.enter_context(tc.tile_pool(name=f"f{s}", bufs=2))
                    fps = ph.enter_context(
                        tc.tile_pool(name=f"fp{s}", bufs=4, space="PSUM"))
                    for t in range(NFT):
                        oh = FT if t < NFT - 1 else H - FT * (NFT - 1)
                        r0 = FT * t
                        # softmax over the 4 head channels
                        es = []
                        for c in range(4):
                            hp = fp.tile([FT, 512], F32, tag="hp")
                            _dma(nc, out=hp[0:oh, :],
                                 in_=hpre[4 * s + c, r0:r0 + oh, 0:512])
                            e = fp.tile([FT, 512], F32, tag=f"e{c}")
                            nc.scalar.activation(
                                e[0:oh, :], hp[0:oh, :],
                                mybir.ActivationFunctionType.Exp,
                                scale=sball[0:oh, 4 * s + c:4 * s + c + 1])
                            es.append(e)
                        ssum = fp.tile([FT, 512], F32, tag="ssum")
                        nc.vector.tensor_add(ssum[0:oh, :], es[0][0:oh, :], es[1][0:oh, :])
                        nc.vector.tensor_add(ssum[0:oh, :], ssum[0:oh, :], es[2][0:oh, :])
                        nc.vector.tensor_add(ssum[0:oh, :], ssum[0:oh, :], es[3][0:oh, :])
                        rec = fp.tile([FT, 512], F32, tag="rec")
                        nc.vector.reciprocal(rec[0:oh, :], ssum[0:oh, :])
                        for c in range(4):
                            nc.vector.tensor_mul(es[c][0:oh, :], es[c][0:oh, :],
                                                 rec[0:oh, :])
                        for c in range(3):
                            ct = fp.tile([128, PADW], F32, tag="cur")
                            _dma(nc, out=ct[:], in_=cur[c, r0:r0 + 128, 0:PADW])
                            xterm = fp.tile([FT, 512], F32, tag="xterm")
                            _dma(nc, out=xterm[0:oh, :],
                                 in_=cur[c, r0 + 12:r0 + 12 + oh, 12:524])
                            S = fp.tile([128, PADW + 1], F32, tag="S")
                            nc.vector.memset(S[:, 0:1], 0.0)
                            nc.vector.tensor_tensor_scan(
                                out=S[:, 1:PADW + 1], data0=ct[:], data1=ct[:],
                                initial=0.0, op0=mybir.AluOpType.add,
                                op1=mybir.AluOpType.bypass)
                            mps = []
                            for ki, (k, r) in enumerate(BLURS):
                                hb = fp.tile([128, 512], F32R, tag=f"hb{ki}")
                                nc.gpsimd.tensor_sub(
                                    hb[:], S[:, 13 + r:13 + r + 512],
                                    S[:, 12 - r:12 - r + 512])
                                mp = fps.tile([FT, 512], F32)
                                nc.tensor.matmul(
                                    mp[0:oh, :], bnd_t[ki][0:128, 0:oh],
                                    hb[:], start=True, stop=True)
                                mps.append(mp)
                            acc = fp.tile([FT, 512], F32, tag="acc")
                            nc.vector.tensor_mul(acc[0:oh, :], es[0][0:oh, :],
                                                 xterm[0:oh, :])
                            for ki in range(2):
                                tmp = fp.tile([FT, 512], F32, tag="tmp")
                                nc.vector.tensor_mul(tmp[0:oh, :], mps[ki][0:oh, :],
                                                     es[ki + 1][0:oh, :])
                                nc.vector.tensor_add(acc[0:oh, :], acc[0:oh, :],
                                                     tmp[0:oh, :])
                            tmp = fp.tile([FT, 512], F32, tag="tmp")
                            nc.vector.tensor_mul(tmp[0:oh, :], mps[2][0:oh, :],
                                                 es[3][0:oh, :])
                            if dstb is not None:
                                nc.vector.tensor_add(acc[0:oh, :], acc[0:oh, :],
                                                     tmp[0:oh, :])
                                _dma(nc, out=dstb[c, 12 + r0:12 + r0 + oh, 12:524],
                                     in_=acc[0:oh, :])
                            elif _OUT_DT == "f32":
                                nc.vector.tensor_add(acc[0:oh, :], acc[0:oh, :],
                                                     tmp[0:oh, :])
                                _dma(nc, out=out[c, r0:r0 + oh, 0:512],
                                     in_=acc[0:oh, :])
                            else:
                                # write the 2-byte result tile directly and
                                # ship it as packed f32 words (16-bit DMAs
                                # and DRAM park+reload both corrupt)
                                nc.vector.tensor_add(acc[0:oh, :], acc[0:oh, :],
                                                     tmp[0:oh, :])
                                accf = fp.tile([FT, 512],
                                               BF16 if _OUT_DT == "bf16" else F16,
                                               tag="accf")
                                nc.scalar.activation(
                                    accf[0:oh, :], acc[0:oh, :],
                                    mybir.ActivationFunctionType.Copy)
                                _dma(nc, out=out[c, r0:r0 + oh, 0:256],
                                     in_=accf[0:oh, :].bitcast(F32))
            if dbg:
                nc.sync.dma_start(out=d_fu1[:], in_=fu1[:])

    orig = nc.to_json_bytes
    nc.to_json_bytes = lambda: _split_waits_json(orig())
    return nc


# ------------------------------------------------------------------ host side
def _prep_weights(inputs, dt_conv):
    import ml_dtypes
    bw1, bw2, bw3 = inputs["bw1"], inputs["bw2"], inputs["bw3"]
    cvt = (lambda a: a.astype(np.float32)) if dt_conv == "f32r" else \
          (lambda a: a.astype(ml_dtypes.bfloat16))
    w1 = np.zeros((27, 32), np.float32)
    for t, (di, dj) in enumerate(_taps()):
        w1[3 * t:3 * t + 3, :] = bw1[:, :, di + 1, dj + 1].T
    w2 = np.zeros((3, 96, 64), np.float32)
    w3 = np.zeros((2, 3, 96, 32), np.float32)
    wh = np.zeros((3, 96, 12), np.float32)
    for j in range(3):
        for g in range(3):
            w2[j, 32 * g:32 * g + 32, :] = bw2[:, :, g, j].T
            for hf in range(2):
                w3[hf, j, 32 * g:32 * g + 32, :] = \
                    bw3[:, 32 * hf:32 * hf + 32, g, j].T
            for h in range(3):
                wh[j, 32 * g:32 * g + 32, 4 * h:4 * h + 4] = \
                    inputs[f"h{h + 1}w"][:, :, g, j].T
    c1t = np.stack([(inputs[f"h{h + 1}c1"][:, :, 0, 0].T / NPIX) for h in range(3)])
    c2t = np.stack([inputs[f"h{h + 1}c2"][:, :, 0, 0].T for h in range(3)])
    bnd = np.zeros((3, 128, FT), np.float32)
    for ki, (k, r) in enumerate(BLURS):
        for i in range(FT):
            lo, hi = i + 12 - r, i + 12 + r
            bnd[ki, lo:hi + 1, i] = 1.0 / (k * k)
    return {
        "w1": w1, "w2": cvt(w2), "w3": cvt(w3), "wh": cvt(wh),
        "bb1": inputs["bb1"].reshape(32, 1).astype(np.float32),
        "bb2": inputs["bb2"].reshape(64, 1).astype(np.float32),
        "bb3": inputs["bb3"].reshape(32, 1).astype(np.float32),
        "hbb": np.concatenate([inputs[f"h{h + 1}b"] for h in range(3)])
               .reshape(12, 1).astype(np.float32),
        "c1t": c1t.astype(np.float32), "c2t": c2t.astype(np.float32),
        "bnd": bnd,
    }


_CACHE = {}
N_CORES = 8


class _Runner:
    """Compile the Bass module to a NEFF-backed jitted callable ONCE and keep
    it cached; steady-state calls are pure dispatch (no retrace, no walrus
    recompile — run_bass_kernel_spmd would redo the full BIR->NEFF compile on
    every invocation)."""

    def __init__(self, nc, n_cores):
        import jax
        import jax.numpy as jnp
        from jax.experimental.shard_map import shard_map
        from jax.sharding import Mesh, NamedSharding, PartitionSpec

        from concourse import bass2jax, mybir as _mybir

        _install_neff_cache()
        bass2jax.install_neuronx_cc_hook()
        self.nc = nc
        self.n_cores = n_cores
        assert nc.dbg_addr is None, "debug path not supported by cached runner"
        partition_name = (nc.partition_id_tensor.name
                          if nc.partition_id_tensor else None)
        in_names, out_names, out_avals = [], [], []
        for alloc in nc.m.functions[0].allocations:
            if not isinstance(alloc, _mybir.MemoryLocationSet):
                continue
            name = alloc.memorylocations[0].name
            if alloc.kind == "ExternalInput":
                if name != partition_name:
                    in_names.append(name)
            elif alloc.kind == "ExternalOutput":
                shape = tuple(alloc.tensor_shape)
                dtype = _mybir.dt.np(alloc.dtype)
                out_avals.append(jax.core.ShapedArray(shape, dtype))
                out_names.append(name)
        self.param_names = list(in_names)
        self.out_names = list(out_names)
        self.out_avals = out_avals
        n_params = len(in_names)
        all_in = list(in_names) + list(out_names)
        if partition_name is not None:
            all_in.append(partition_name)

        def _body(*args):
            operands = list(args)
            if partition_name is not None:
                operands.append(bass2jax.partition_id_tensor())
            outs = bass2jax._bass_exec_p.bind(
                *operands,
                out_avals=tuple(out_avals),
                in_names=tuple(all_in),
                out_names=tuple(out_names),
                lowering_input_output_aliases=(),
                sim_require_finite=True,
                sim_require_nnan=True,
                nc=nc,
            )
            return tuple(outs)

        devices = jax.devices()[:n_cores]
        assert len(devices) == n_cores
        mesh = Mesh(np.asarray(devices), ("core",))
        nin = n_params + len(out_names)
        # No donation: the carrier params are dead inputs (the NEFF binds
        # only its own output buffers), so persistent on-device zeros can be
        # reused every call — no per-call zeros executable or transfer.
        self.sharded = jax.jit(
            shard_map(_body, mesh=mesh,
                      in_specs=(PartitionSpec("core"),) * nin,
                      out_specs=(PartitionSpec("core"),) * len(out_names),
                      check_rep=False),
            keep_unused=True)
        shard_spec = NamedSharding(mesh, PartitionSpec("core"))
        self._carriers = [
            jax.device_put(
                np.zeros((n_cores * a.shape[0], *a.shape[1:]), a.dtype),
                shard_spec)
            for a in out_avals]

    def __call__(self, in_concat: dict) -> list:
        args = [in_concat[name] for name in self.param_names]
        args.extend(self._carriers)
        outs = self.sharded(*args)
        return {name: outs[i] for i, name in enumerate(self.out_names)}


def _kernel_single_client(inputs, x, a_vals) -> np.ndarray:
    key = (DT_CONV, tuple(a_vals))
    if key not in _CACHE:
        _CACHE[key] = _Runner(build_nc(a_vals, DT_CONV), N_CORES)
    runner = _CACHE[key]
    shared = _prep_weights(inputs, DT_CONV)
    # global (core-concatenated) input layout: x reshapes for free, the small
    # replicated weights tile 8x
    concat = {"x": x.reshape(N_CORES * 3, H, W)}
    for k, v in shared.items():
        concat[k] = np.concatenate([v] * N_CORES, axis=0)
    out = np.asarray(runner(concat)["out"])
    if _OUT_DT != "f32":
        import ml_dtypes
        pair_dt = ml_dtypes.bfloat16 if _OUT_DT == "bf16" else np.float16
        out = out.view(pair_dt)
    return out.astype(np.float32).reshape(N_CORES, 3, H, W)


# ---------------------------------------------------- multi-process workers
# The axon tunnel is ~45MB/s per client connection but scales with extra
# client PROCESSES. 8 workers (one NeuronCore each) move their 1.5MB f16
# x-slice + 3MB f32 out-slice concurrently: ~120ms/call vs ~850ms through
# one connection. IPC is via SharedMemory + stdin/stdout lines.

_W_XBYTES = 3 * H * W * 2          # f16 x slice per worker
_W_OBYTES = 3 * H * W * 4          # f32 out slice per worker
_W_WBYTES = 4 << 20                # weights blob (pickled dict, <4MB)


def _shm_layout(n):
    return {
        "x": (0, n * _W_XBYTES),
        "out": (n * _W_XBYTES, n * _W_OBYTES),
        "w": (n * (_W_XBYTES + _W_OBYTES), _W_WBYTES),
    }


def _worker_main():
    import pickle
    from multiprocessing import shared_memory

    w = int(os.environ["KW_IDX"])
    shm = shared_memory.SharedMemory(name=os.environ["KW_SHM"])
    lay = _shm_layout(N_CORES)
    a_vals = [float(v) for v in os.environ["KW_AVALS"].split(",")]
    bir_path = os.environ["KW_BIR"]

    import jax

    dev = jax.devices()[w]
    _install_neff_cache()
    from concourse import bass2jax
    bass2jax.install_neuronx_cc_hook()

    nc = build_nc(a_vals, DT_CONV)
    if w == 0:
        bir = nc.to_json_bytes()
        with open(bir_path + ".part", "wb") as f:
            f.write(bir)
        os.replace(bir_path + ".part", bir_path)
    else:
        import time as _t
        for _ in range(7200):
            if os.path.exists(bir_path):
                break
            _t.sleep(0.5)
        bir = open(bir_path, "rb").read()
    # all workers embed worker 0's BIR bytes -> one walrus compile, shared
    # through the disk NEFF cache
    nc.to_json_bytes = lambda: bir

    from concourse import mybir as _mybir
    assert nc.dbg_addr is None
    partition_name = (nc.partition_id_tensor.name
                      if nc.partition_id_tensor else None)
    in_names, out_names, out_avals = [], [], []
    for alloc in nc.m.functions[0].allocations:
        if not isinstance(alloc, _mybir.MemoryLocationSet):
            continue
        name = alloc.memorylocations[0].name
        if alloc.kind == "ExternalInput":
            if name != partition_name:
                in_names.append(name)
        elif alloc.kind == "ExternalOutput":
            out_avals.append(jax.core.ShapedArray(
                tuple(alloc.tensor_shape), _mybir.dt.np(alloc.dtype)))
            out_names.append(name)
    all_in = in_names + out_names
    if partition_name is not None:
        all_in.append(partition_name)

    def _body(*args):
        operands = list(args)
        if partition_name is not None:
            operands.append(bass2jax.partition_id_tensor())
        return tuple(bass2jax._bass_exec_p.bind(
            *operands, out_avals=tuple(out_avals), in_names=tuple(all_in),
            out_names=tuple(out_names), lowering_input_output_aliases=(),
            sim_require_finite=True, sim_require_nnan=True, nc=nc))

    jfn = jax.jit(_body, keep_unused=True)
    carriers = [jax.device_put(np.zeros(a.shape, a.dtype), dev)
                for a in out_avals]

    xo, _ = lay["x"]
    oo, _ = lay["out"]
    wo, _ = lay["w"]
    x16 = np.ndarray((3, H, W), np.float16,
                     buffer=shm.buf[xo + w * _W_XBYTES:
                                    xo + (w + 1) * _W_XBYTES])
    oview = np.ndarray((3, H, W), np.float32,
                       buffer=shm.buf[oo + w * _W_OBYTES:
                                      oo + (w + 1) * _W_OBYTES])

    # warm compile + one exec
    wlen = int.from_bytes(bytes(shm.buf[wo:wo + 8]), "little")
    shared = pickle.loads(bytes(shm.buf[wo + 8:wo + 8 + wlen]))
    args = [jax.device_put(shared[nm] if nm != "x" else np.asarray(x16), dev)
            for nm in in_names]
    outs = jfn(*args, *carriers)
    np.asarray(outs[0])
    print("READY", flush=True)

    for line in sys.stdin:
        line = line.strip()
        if not line:
            continue
        if line.startswith("QUIT"):
            break
        seq = line.split()[1]
        wlen = int.from_bytes(bytes(shm.buf[wo:wo + 8]), "little")
        shared = pickle.loads(bytes(shm.buf[wo + 8:wo + 8 + wlen]))
        args = [jax.device_put(shared[nm] if nm != "x" else np.asarray(x16),
                               dev) for nm in in_names]
        outs = jfn(*args, *carriers)
        res = np.asarray(outs[0])
        if _OUT_DT != "f32":
            import ml_dtypes
            pair = ml_dtypes.bfloat16 if _OUT_DT == "bf16" else np.float16
            res = res.view(pair).astype(np.float32).reshape(3, H, W)
        oview[...] = res
        print(f"DONE {seq}", flush=True)


class _WorkerPool:
    def __init__(self, a_vals):
        import pickle
        import subprocess
        import tempfile
        from multiprocessing import shared_memory

        lay = _shm_layout(N_CORES)
        total = sum(sz for _, sz in lay.values())
        self.shm = shared_memory.SharedMemory(create=True, size=total)
        self.lay = lay
        self.seq = 0
        bir_path = tempfile.mktemp(prefix="kernel_bir_")
        kdir = os.path.dirname(os.path.abspath(__file__))
        env = dict(os.environ,
                   KW_SHM=self.shm.name,
                   KW_AVALS=",".join(str(v) for v in a_vals),
                   KW_BIR=bir_path,
                   PYTHONPATH=kdir + os.pathsep + os.environ.get(
                       "PYTHONPATH", ""))
        self.procs = []
        for w in range(N_CORES):
            e = dict(env, KW_IDX=str(w))
            p = subprocess.Popen(
                [sys.executable, "-c",
                 "import kernel; kernel._worker_main()"],
                stdin=subprocess.PIPE, stdout=subprocess.PIPE,
                stderr=open(f"/tmp/kw{w}.log", "w"), env=e, cwd=kdir,
                text=True)
            self.procs.append(p)

    def wait_ready(self):
        for p in self.procs:
            line = p.stdout.readline()
            if "READY" not in line:
                raise RuntimeError(f"worker failed: {line!r}")

    def put_weights(self, shared):
        import pickle
        blob = pickle.dumps(shared, protocol=4)
        wo, wsz = self.lay["w"]
        assert len(blob) + 8 <= wsz
        self.shm.buf[wo:wo + 8] = len(blob).to_bytes(8, "little")
        self.shm.buf[wo + 8:wo + 8 + len(blob)] = blob

    def run(self, x16_all):
        xo, _ = self.lay["x"]
        xbytes = x16_all.tobytes()
        self.shm.buf[xo:xo + len(xbytes)] = xbytes
        self.seq += 1
        for p in self.procs:
            p.stdin.write(f"GO {self.seq}\n")
            p.stdin.flush()
        for p in self.procs:
            line = p.stdout.readline()
            if f"DONE {self.seq}" not in line:
                raise RuntimeError(f"worker desync: {line!r}")
        oo, osz = self.lay["out"]
        out = np.ndarray((N_CORES, 3, H, W), np.float32,
                         buffer=self.shm.buf[oo:oo + osz]).copy()
        return out

    def alive(self):
        return all(p.poll() is None for p in self.procs)


_POOL = {}


def kernel(**inputs) -> np.ndarray:
    inputs = {k: np.asarray(v) for k, v in inputs.items()}
    x = np.asarray(inputs["x"]).astype(np.float16 if _X_F16 else np.float32)
    a_vals = [float(np.asarray(inputs[f"a{i}"]).reshape(-1)[0]) for i in (1, 2, 3)]
    if os.environ.get("KERNEL_WORKERS") != "1" or _POOL.get("failed"):
        return _kernel_single_client(inputs, x, a_vals)
    key = tuple(a_vals)
    try:
        if key not in _POOL:
            pool = _WorkerPool(a_vals)
            pool.put_weights(_prep_weights(inputs, DT_CONV))
            pool.wait_ready()
            _POOL[key] = pool
        pool = _POOL[key]
        if not pool.alive():
            raise RuntimeError("worker died")
        pool.put_weights(_prep_weights(inputs, DT_CONV))
        return pool.run(x)
    except Exception:
        _POOL.pop(key, None)
        _POOL["failed"] = True
        return _kernel_single_client(inputs, x, a_vals)

